# revision 63
# baseline (speedup 1.0000x reference)
"""Trainium2 Bass kernel for CarlosSelfAttention (B=2, T=2048, C=1024, H=16).

Sharding: tensor-parallel over heads. 8 cores x 2 heads each.
Each core computes q/k/v projections for its 2 heads, RoPE, causal
attention, and a partial out-projection against its 128 columns of Wo.
The host sums the 8 partial outputs (the TP all-reduce) and adds the
output bias plus the (v-bias @ Wo.T) correction term.

v2: full bf16 pipeline (matmuls, activations, DMA) with a dense PE
schedule: qkv for batch 1 is interleaved into attention for batch 0 as
"filler" matmuls, and both out-projections are interleaved into
attention for batch 1, so the PE never idles long enough for the HAM
clock gate to re-throttle. V^T is produced directly by x-stationary
matmuls (no PE transposes). All on-chip layouts are "transposed"
([dim, token]) so every matmul contraction lands on the partition axis.
"""

import numpy as np

import concourse.bass as bass
import concourse.tile as tile
from concourse import bacc, mybir
from concourse.bass_utils import run_bass_kernel_spmd

F32 = mybir.dt.float32
BF16 = mybir.dt.bfloat16
AF = mybir.ActivationFunctionType

B, T, C, H, HD = 2, 2048, 1024, 16, 64
NCORES = 8
TB = B * T          # 4096
QCH = 512           # q-chunk (moving dim)
NQC = T // QCH      # 4 q-chunks per batch
NKT = T // 128      # 16 k-tiles per batch
NTC = TB // QCH     # 8 t-chunks for the projections
NCT = C // 128      # 8 contraction tiles
VW = HD + 1         # 65: value dims + ones row for the softmax sum

_PROG_CACHE: dict = {}


def _emit(tc, mode, dram, debug=None):
    nc = tc.nc
    from contextlib import ExitStack
    from collections import deque

    xT, wqk, wv, bqk, cosT, sinS, woT, y = (
        dram["xT"], dram["wqk"], dram["wv"], dram["bqk"], dram["cosT"],
        dram["sinS"], dram["woT"], dram["y"])
    maskT = dram.get("maskT")

    with ExitStack() as ctx:
        constp = ctx.enter_context(tc.tile_pool(name="const", bufs=1))
        pers = ctx.enter_context(tc.tile_pool(name="pers", bufs=1))

        # ---- constants (order matters: first matmul needs bqk+wqk only) ----
        wsb = constp.tile([128, NCT, 256], BF16)
        nc.sync.dma_start(wsb[:, :, 0:128],
                          wqk[:, 0:128].rearrange("(a p) m -> p a m", p=128))
        bqk_sb = constp.tile([128, 2], F32)
        wv_sb = constp.tile([128, NCT, 128], BF16)

        def early_consts():
            nc.sync.dma_start(wsb[:, :, 128:256],
                              wqk[:, 128:256].rearrange("(a p) m -> p a m",
                                                        p=128))
            nc.sync.dma_start(bqk_sb[:], bqk[:])

        # ---- persistent activations ----
        qTb = pers.tile([128, TB], BF16)
        kTb = pers.tile([128, TB], BF16)
        Vsb = [[pers.tile([128, NKT * VW], BF16, name=f"Vsb{b}{h}")
                for h in range(2)] for b in range(B)]
        OTb = [pers.tile([128, T], BF16, name=f"OTb{b}") for b in range(B)]
        for b in range(B):
            for h in range(2):
                nc.gpsimd.memset(Vsb[b][h][:], 1.0)
        id65 = constp.tile([VW, VW], BF16)
        nc.gpsimd.memset(id65[:], 1.0)
        nc.gpsimd.affine_select(
            out=id65[:], in_=id65[:], compare_op=mybir.AluOpType.is_equal,
            fill=0.0, base=0, channel_multiplier=1, pattern=[[-1, VW]])

        # cos/sin + wo land while the first qkv matmuls run (DMAs are
        # emitted inside phase A, after the first x-tile loads)
        cos_sb = constp.tile([128, T], BF16)
        sin_sb = constp.tile([128, T], BF16)
        wo_sb = constp.tile([128, C], BF16)
        wo_lo = constp.tile([64, C], BF16)

        def late_consts():
            nc.sync.dma_start(wv_sb[:],
                              wv[:].rearrange("(a p) m -> p a m", p=128))
            nc.sync.dma_start(cos_sb[:], cosT[:])
            nc.sync.dma_start(sin_sb[:], sinS[:])
            nc.sync.dma_start(wo_sb[:], woT[:])
            nc.sync.dma_start(wo_lo[:], woT[64:128, :])
            if mode == "bias":
                nc.sync.dma_start(
                    mask_sb[:], maskT[:].rearrange("(a p) m -> p a m", p=128))
        if mode == "bias":
            mask_sb = pers.tile([128, NKT, T], BF16)

        def qkv_units(xp, psp, swpp, rtp, qfp, tca, tcb, qtag="q", vtag="v",
                      evict_eng="scalar", extra=None):
            """Units for q/k projection + rope + V^T for t-chunks tca, tcb.
            Returns a list of closures; call in order to emit."""
            units = []
            tsa = slice(tca * QCH, (tca + 1) * QCH)
            tsb = slice(tcb * QCH, (tcb + 1) * QCH)
            xx = {}
            qf = {}

            def load(tci, ts, half):
                def u():
                    if half == 0:
                        xx[tci] = xp.tile([128, NCT, QCH], BF16, tag="x",
                                          name=f"xt{tci}")
                    xt = xx[tci]
                    h = NCT // 2
                    hs = slice(0, h) if half == 0 else slice(h, NCT)
                    nc.sync.dma_start(
                        xt[:, hs, :],
                        xT[hs.start * 128:hs.stop * 128, ts]
                        .rearrange("(a p) m -> p a m", p=128))
                return u

            def mm_g(g, ct, pget):
                def u():
                    psa, psb = pget()
                    w = wsb[:, ct, g * 128:(g + 1) * 128]
                    nc.tensor.matmul(psa[:], w, xx[tca][:, ct, :],
                                     start=(ct == 0), stop=(ct == NCT - 1))
                    nc.tensor.matmul(psb[:], w, xx[tcb][:, ct, :],
                                     start=(ct == 0), stop=(ct == NCT - 1))
                return u

            def evict(g, pget, which, ts, tci):
                def u():
                    ps = pget()[which]
                    qf[(g, tci)] = f = qfp.tile([128, QCH], BF16, tag="qf",
                                                name=f"qf{g}_{tci}")
                    if evict_eng == "scalar":
                        nc.scalar.activation(f[:], ps[:], AF.Identity,
                                             bias=bqk_sb[:, g:g + 1])
                    else:
                        nc.vector.tensor_scalar_add(f[:], ps[:],
                                                    bqk_sb[:, g:g + 1])
                return u

            def rope(g, tci):
                def u():
                    f = qf.pop((g, tci))
                    swp = swpp.tile([128, QCH], BF16, tag="swp",
                                    name=f"swp{g}_{tci}")
                    for o in (0, 64):
                        nc.sync.dma_start(swp[o:o + 32, :],
                                          f[o + 32:o + 64, :])
                        nc.sync.dma_start(swp[o + 32:o + 64, :],
                                          f[o:o + 32, :])
                    dst = qTb if g == 0 else kTb
                    # position within the batch (rope tables are [128, T])
                    ps_ = slice((tci % NQC) * QCH, (tci % NQC + 1) * QCH)
                    gs = slice(tci * QCH, (tci + 1) * QCH)
                    rt = rtp.tile([128, QCH], BF16, tag="rt",
                                  name=f"rt{g}_{tci}")
                    nc.vector.tensor_mul(rt[:], swp[:], sin_sb[:, ps_])
                    nc.vector.tensor_mul(dst[:, gs], f[:], cos_sb[:, ps_])
                    nc.vector.tensor_add(dst[:, gs], dst[:, gs], rt[:])
                return u

            def vtr(tci, sub):
                # V^T for token sub-tile: [128 tokens, 128 dims(2 heads)]
                def u():
                    b, tt = divmod(tci * (QCH // 128) + sub, NKT)
                    psv = psp.tile([128, 128], F32, tag=vtag,
                                   name=f"psv{tci}_{sub}")
                    xt = xx[tci]
                    for ct in range(NCT):
                        nc.tensor.matmul(
                            psv[:], xt[:, ct, sub * 128:(sub + 1) * 128],
                            wv_sb[:, ct, :],
                            start=(ct == 0), stop=(ct == NCT - 1))
                    for h in range(2):
                        nc.vector.tensor_copy(
                            Vsb[b][h][:, tt * VW:tt * VW + HD],
                            psv[:, h * HD:(h + 1) * HD])
                return u

            ps_cache = {}

            def pget_for(g):
                def pget():
                    if g not in ps_cache:
                        ps_cache[g] = (
                            psp.tile([128, QCH], F32, tag=qtag,
                                     name=f"psq{tca}_{g}a"),
                            psp.tile([128, QCH], F32, tag=qtag,
                                     name=f"psq{tca}_{g}b"))
                    return ps_cache[g]
                return pget

            # loads first (deep DMA prefetch), then matmuls
            units.append(load(tca, tsa, 0))
            units.append(load(tcb, tsb, 0))
            if extra is not None:
                units.append(extra)
            units.append(load(tca, tsa, 1))
            units.append(load(tcb, tsb, 1))
            for g in range(2):
                pget = pget_for(g)
                for ct in range(NCT):
                    units.append(mm_g(g, ct, pget))
                units.append(evict(g, pget, 0, tsa, tca))
                units.append(evict(g, pget, 1, tsb, tcb))
            for g in range(2):
                units.append(rope(g, tca))
                units.append(rope(g, tcb))
            for tci in (tca, tcb):
                for sub in range(QCH // 128):
                    units.append(vtr(tci, sub))
            return units

        def proj_units(psp, ybp, b, tts, scalar_stride=0):
            """Out-projection units for token-tiles tts of batch b.
            Every scalar_stride-th eviction goes to ScalarE (0 = never).
            Each tile tt: 2 matmuls -> one merged [128, 1024] store."""
            units = []
            ybs = {}

            def unit(tt, ncol, on_scalar):
                def u():
                    ps = psp.tile([128, QCH], F32, tag="f",
                                  name=f"psy{b}_{tt}_{ncol}")
                    nc.tensor.matmul(
                        ps[:], OTb[b][:, tt * 128:(tt + 1) * 128],
                        wo_sb[:, ncol * QCH:(ncol + 1) * QCH],
                        start=True, stop=True)
                    if ncol == 0:
                        ybs[tt] = ybp.tile([128, C], BF16, tag="yb",
                                           name=f"yb{b}_{tt}")
                    yb = ybs[tt]
                    ys = slice(ncol * QCH, (ncol + 1) * QCH)
                    if on_scalar:
                        nc.scalar.activation(yb[:, ys], ps[:], AF.Copy)
                    else:
                        nc.vector.tensor_copy(yb[:, ys], ps[:])
                    if ncol == 1:
                        nc.sync.dma_start(
                            y[b * T + tt * 128:b * T + (tt + 1) * 128, :],
                            ybs.pop(tt)[:])
                return u
            i = 0
            for tt in tts:
                for ncol in range(2):
                    i += 1
                    units.append(unit(tt, ncol,
                                      scalar_stride and i % scalar_stride == 0))
            return units

        def attn_b(pools, b, fillers, qc_done=None, dbg=None, tail_out=None):
            pss, pso, ptp, smol, bcp = pools
            PIPE = 3
            nks = [4 * (qc + 1) if mode == "causal" else NKT
                   for qc in range(NQC)]
            kts_left = sum(nks)
            for qc in range(NQC):
                nk = nks[qc]
                qs = slice(b * T + qc * QCH, b * T + (qc + 1) * QCH)
                psO0 = pso.tile([VW, QCH], F32, tag="o", name=f"psO0_{b}{qc}")
                psO1 = pso.tile([VW, QCH], F32, tag="o", name=f"psO1_{b}{qc}")
                pts = {}

                def emit_pv(j, nk=nk, psO0=psO0, psO1=psO1, pts=pts):
                    st, sp = (j == 0), (j == nk - 1)
                    pt = pts.pop(j)
                    nc.tensor.matmul(
                        psO0[:], Vsb[b][0][:, j * VW:(j + 1) * VW],
                        pt[:, 0:QCH], start=st, stop=sp)
                    nc.tensor.matmul(
                        psO1[:], Vsb[b][1][:, j * VW:(j + 1) * VW],
                        pt[:, QCH:2 * QCH], start=st, stop=sp)

                for kt in range(nk):
                    ks = slice(b * T + kt * 128, b * T + (kt + 1) * 128)
                    psS = pss.tile([128, 2 * QCH], F32, tag="s",
                                   name=f"psS{b}{qc}{kt}")
                    nc.tensor.matmul(psS[:, 0:QCH], kTb[0:64, ks],
                                     qTb[0:64, qs], start=True, stop=True)
                    nc.tensor.matmul(psS[:, QCH:2 * QCH], kTb[64:128, ks],
                                     qTb[64:128, qs], start=True, stop=True)
                    pt = ptp.tile([128, 2 * QCH], BF16, tag="pt",
                                  name=f"pt{b}{qc}{kt}")
                    nc.scalar.activation(pt[:], psS[:], AF.Exp)
                    if mode == "causal" and kt >= 4 * qc:
                        base = qc * QCH - kt * 128
                        ptv = pt[:].rearrange("p (h q) -> p h q", q=QCH)
                        nc.gpsimd.affine_select(
                            out=ptv, in_=ptv,
                            compare_op=mybir.AluOpType.is_ge,
                            fill=0.0, base=base, channel_multiplier=-1,
                            pattern=[[0, 2], [1, QCH]])
                    elif mode == "bias":
                        mt = mask_sb[:, kt, qc * QCH:(qc + 1) * QCH]
                        nc.vector.tensor_mul(pt[:, 0:QCH], pt[:, 0:QCH], mt)
                        nc.vector.tensor_mul(pt[:, QCH:2 * QCH],
                                             pt[:, QCH:2 * QCH], mt)
                    pts[kt] = pt
                    if dbg is not None and qc == 0 and kt == 0:
                        nc.vector.tensor_copy(dbg["pt"][:], pt[:])
                    # pace the fillers across ALL remaining k-tiles
                    want = (len(fillers) + kts_left - 1) // kts_left \
                        if fillers else 0
                    kts_left -= 1
                    for _ in range(min(want, 4)):
                        if fillers:
                            fillers.popleft()()
                    if kt >= PIPE:
                        emit_pv(kt - PIPE)
                for j in range(max(0, nk - PIPE), nk):
                    emit_pv(j)

                # fast-evict psO to SBUF (frees the PSUM banks in one DVE
                # round-trip), then normalize from the SBUF copy
                oqs = slice(qc * QCH, (qc + 1) * QCH)
                nm = f"{b}{qc}"
                if dbg is not None and qc == 0:
                    nc.vector.tensor_copy(dbg["po"][0:VW, :], psO0[:])
                if tail_out is not None and qc == NQC - 1:
                    # Final q-chunk: skip normalize/OTb entirely.  Evict the
                    # unnormalized output (bf16) + per-token reciprocal sums;
                    # the tail projection folds the softmax normalization in
                    # as a per-token (per-partition) scale.
                    occ = smol.tile([VW, 2 * QCH], BF16, tag="occ",
                                    name="octail")
                    nc.scalar.activation(occ[0:VW, 0:QCH], psO0[:], AF.Copy)
                    nc.scalar.activation(occ[0:VW, QCH:2 * QCH], psO1[:],
                                         AF.Copy)
                    tail_out["oc"] = occ
                    if qc_done is not None:
                        qc_done(qc)
                    continue
                oc = smol.tile([VW, 2 * QCH], F32, tag="oc", name=f"oc{nm}")
                if qc == NQC - 1:
                    # scalar is idle at the window end; keep DVE clear for
                    # the latency-critical chain to the final projections
                    nc.scalar.activation(oc[0:VW, 0:QCH], psO0[:], AF.Copy)
                    nc.scalar.activation(oc[0:VW, QCH:2 * QCH], psO1[:],
                                         AF.Copy)
                else:
                    nc.vector.tensor_copy(oc[0:VW, 0:QCH], psO0[:])
                    nc.vector.tensor_copy(oc[0:VW, QCH:2 * QCH], psO1[:])
                rz = smol.tile([1, 2 * QCH], F32, tag="rz", name=f"rz{nm}")
                nc.gpsimd.dma_start(rz[:], oc[64:65, :])
                rr = smol.tile([1, 2 * QCH], F32, tag="rr", name=f"rr{nm}")
                nc.vector.reciprocal_approx_fast(rr[:], rz[:])
                bc = bcp.tile([64, 2 * QCH], F32, tag="bc", name=f"bc{nm}")
                nc.gpsimd.partition_broadcast(bc[:], rr[:])
                if dbg is not None and qc == 0:
                    nc.vector.tensor_copy(dbg["nr"][0:64, :], bc[:])
                    nc.vector.tensor_copy(dbg["nr2"][0:1, :], rr[:])
                    nc.vector.tensor_copy(dbg["nr2"][64:65, :], oc[64:65, :])
                nc.vector.tensor_mul(OTb[b][0:64, oqs], oc[0:64, 0:QCH],
                                     bc[:, 0:QCH])
                otmp = bcp.tile([64, QCH], BF16, tag="otmp", name=f"ot{nm}")
                nc.vector.tensor_mul(otmp[:], oc[0:64, QCH:2 * QCH],
                                     bc[:, QCH:2 * QCH])
                nc.gpsimd.dma_start(OTb[b][64:128, oqs], otmp[:])
                if qc_done is not None:
                    qc_done(qc)
            while fillers:
                fillers.popleft()()

        # ---- phase A: qkv + rope + V^T for batch 0 ----
        with tc.tile_pool(name="xp", bufs=5) as xp, \
             tc.tile_pool(name="qfp", bufs=6) as qfp, \
             tc.tile_pool(name="swp", bufs=4) as swpp, \
             tc.tile_pool(name="rtp", bufs=4) as rtp:
            with tc.tile_pool(name="psA", bufs=4, space="PSUM") as psA:
                def phaseA_consts():
                    early_consts()
                    late_consts()
                for u in qkv_units(xp, psA, swpp, rtp, qfp, 0, 1,
                                   extra=phaseA_consts):
                    u()
                for u in qkv_units(xp, psA, swpp, rtp, qfp, 2, 3):
                    u()

            # ---- phases B/C: attention with fillers ----
            with tc.tile_pool(name="pss", bufs=2, space="PSUM") as pss, \
                 tc.tile_pool(name="pso", bufs=2, space="PSUM") as pso, \
                 tc.tile_pool(name="psf", bufs=2, space="PSUM") as psf, \
                 tc.tile_pool(name="ptp", bufs=8) as ptp, \
                 tc.tile_pool(name="smol", bufs=2) as smol, \
                 tc.tile_pool(name="bcp", bufs=2) as bcp, \
                 tc.tile_pool(name="ybp", bufs=4) as ybp:
                dbg = None
                if debug is not None:
                    dbg = {"pt": pers.tile([128, 2 * QCH], BF16, name="dbgpt"),
                           "po": pers.tile([128, QCH], F32, name="dbgpo"),
                           "nr": pers.tile([128, 2 * QCH], F32, name="dbgnr"),
                           "nr2": pers.tile([128, 2 * QCH], F32,
                                            name="dbgnr2")}
                fillers = deque()
                fillers.extend(qkv_units(xp, psf, swpp, rtp, qfp, 4, 5,
                                         qtag="f", vtag="f",
                                         evict_eng="vector"))
                fillers.extend(qkv_units(xp, psf, swpp, rtp, qfp, 6, 7,
                                         qtag="f", vtag="f",
                                         evict_eng="vector"))
                attn_b((pss, pso, ptp, smol, bcp), 0, fillers, dbg=dbg)

                fillers = deque()
                fillers.extend(proj_units(psf, ybp, 0, range(NKT - 2),
                                          scalar_stride=4))

                def qc_done(qc):
                    if qc > 0:
                        fillers.extend(proj_units(
                            psf, ybp, 1, range(4 * (qc - 1), 4 * qc),
                            scalar_stride=4))
                tail = {}
                attn_b((pss, pso, ptp, smol, bcp), 1, fillers, qc_done,
                       tail_out=tail)
                # held-back b0 tiles fill the PE while the tail sums/recip
                # chain completes
                for u in proj_units(psf, ybp, 0, range(NKT - 2, NKT),
                                    scalar_stride=2):
                    u()
                # extract per-token softmax sums via tiny PE transposes,
                # then reciprocal on token partitions
                occ = tail["oc"]
                rsf = smol.tile([128, 8], F32, tag="rsf", name="rsftail")
                for o in range(8):
                    pst = psf.tile([128, VW], BF16, tag="f", name=f"pst{o}")
                    nc.tensor.transpose(pst[:], occ[0:VW, o * 128:(o + 1) * 128],
                                        id65[:])
                    nc.vector.tensor_copy(rsf[:, o:o + 1], pst[:, 64:65])
                rc = smol.tile([128, 8], F32, tag="rc", name="rctail")
                nc.vector.reciprocal_approx_fast(rc[:], rsf[:])
                qc0 = NQC - 1
                for tr in range(4):
                    tt = 4 * qc0 + tr
                    ts0 = slice(tr * 128, (tr + 1) * 128)
                    ts1 = slice(QCH + tr * 128, QCH + (tr + 1) * 128)
                    ybt = ybp.tile([128, C], BF16, tag="yb", name=f"ybt{tt}")
                    for ncol in range(2):
                        ys = slice(ncol * QCH, (ncol + 1) * QCH)
                        ps0 = psf.tile([128, QCH], F32, tag="f",
                                       name=f"tps0_{tt}_{ncol}")
                        nc.tensor.matmul(ps0[:], occ[0:64, ts0],
                                         wo_sb[0:64, ys],
                                         start=True, stop=True)
                        ps1 = psf.tile([128, QCH], F32, tag="f",
                                       name=f"tps1_{tt}_{ncol}")
                        nc.tensor.matmul(ps1[:], occ[0:64, ts1],
                                         wo_lo[0:64, ys],
                                         start=True, stop=True)
                        th = bcp.tile([128, QCH], F32, tag="th",
                                      name=f"th{tt}_{ncol}")
                        nc.scalar.activation(th[:], ps1[:], AF.Copy,
                                             scale=rc[:, 4 + tr:5 + tr])
                        nc.vector.scalar_tensor_tensor(
                            ybt[:, ys], ps0[:], rc[:, tr:tr + 1], th[:],
                            mybir.AluOpType.mult, mybir.AluOpType.add)
                    nc.sync.dma_start(
                        y[T + tt * 128:T + (tt + 1) * 128, :], ybt[:])

        if debug is not None:
            nc.sync.dma_start(debug["dbg_pt"][:], dbg["pt"][:])
            nc.sync.dma_start(debug["dbg_po"][:], dbg["po"][:])
            nc.sync.dma_start(debug["dbg_nr"][:], dbg["nr"][:])
            nc.sync.dma_start(debug["dbg_nr2"][:], dbg["nr2"][:])
            nc.sync.dma_start(debug["dbg_q"][:], qTb[:])
            nc.sync.dma_start(debug["dbg_k"][:], kTb[:])
            nc.sync.dma_start(debug["dbg_v00"][:], Vsb[0][0][:])
            nc.sync.dma_start(debug["dbg_v01"][:], Vsb[1][0][:])
            nc.sync.dma_start(debug["dbg_o0"][:], OTb[0][:])
            nc.sync.dma_start(debug["dbg_o1"][:], OTb[1][:])


def _build_program(mode):
    if mode in _PROG_CACHE:
        return _PROG_CACHE[mode]
    nc = bacc.Bacc("TRN2", target_bir_lowering=False, debug=False,
                   num_devices=NCORES)
    dram = {
        "xT": nc.dram_tensor("xT", [C, TB], BF16, kind="ExternalInput").ap(),
        "wqk": nc.dram_tensor("wqk", [C, 256], BF16, kind="ExternalInput").ap(),
        "wv": nc.dram_tensor("wv", [C, 128], BF16, kind="ExternalInput").ap(),
        "bqk": nc.dram_tensor("bqk", [128, 2], F32, kind="ExternalInput").ap(),
        "cosT": nc.dram_tensor("cosT", [128, T], BF16,
                               kind="ExternalInput").ap(),
        "sinS": nc.dram_tensor("sinS", [128, T], BF16,
                               kind="ExternalInput").ap(),
        "woT": nc.dram_tensor("woT", [128, C], BF16, kind="ExternalInput").ap(),
        "y": nc.dram_tensor("y", [TB, C], BF16, kind="ExternalOutput").ap(),
    }
    if mode == "bias":
        dram["maskT"] = nc.dram_tensor("maskT", [T, T], BF16,
                                       kind="ExternalInput").ap()
    with tile.TileContext(nc) as tc:
        _emit(tc, mode, dram)
    nc.compile()
    _PROG_CACHE[mode] = (nc, dram)
    return nc, dram


def _rope_tables():
    inv_freq = 1.0 / (10000.0 ** (np.arange(0, HD, 2, dtype=np.float64) / HD))
    freqs = np.arange(T, dtype=np.float64)[:, None] * inv_freq[None, :]
    cos = np.concatenate([np.cos(freqs), np.cos(freqs)], axis=-1)  # [T, 64]
    sin = np.concatenate([np.sin(freqs), np.sin(freqs)], axis=-1)
    cE = cos[:, 0::2].T  # [32, T] rows i -> dim 2i
    cO = cos[:, 1::2].T
    sE = sin[:, 0::2].T
    sO = sin[:, 1::2].T
    cosT = np.concatenate([cE, cO, cE, cO], axis=0)
    sinS = np.concatenate([-sE, sO, -sE, sO], axis=0)
    return cosT, sinS


def _bf16(a):
    import ml_dtypes
    return np.ascontiguousarray(a.astype(ml_dtypes.bfloat16))


def _detect_mode(mask):
    mb = np.asarray(mask).reshape(T, T)
    if np.array_equal(mb != 0, np.tril(np.ones((T, T), dtype=bool))):
        return "causal", mb
    if np.all(mb != 0):
        return "dense", mb
    return "bias", mb


def _prepare_in_maps(x, mask, Wqkv, bqkv, Wo, bo, mode, mb):
    x = np.asarray(x, dtype=np.float32)
    Wqkv = np.asarray(Wqkv, dtype=np.float32)
    bqkv = np.asarray(bqkv, dtype=np.float32)
    Wo = np.asarray(Wo, dtype=np.float32)

    xTn = _bf16(x.reshape(TB, C).T)
    cosT, sinS = _rope_tables()
    cosT, sinS = _bf16(cosT), _bf16(sinS)
    scale = 1.0 / np.sqrt(np.float32(HD))

    evens = np.arange(0, HD, 2)
    odds = evens + 1

    in_maps = []
    for c in range(NCORES):
        h0, h1 = 2 * c, 2 * c + 1
        qrows = np.concatenate([h0 * HD + evens, h0 * HD + odds,
                                h1 * HD + evens, h1 * HD + odds])
        krows = C + qrows
        vrows = np.concatenate([2 * C + h0 * HD + np.arange(HD),
                                2 * C + h1 * HD + np.arange(HD)])
        wq = Wqkv[qrows, :] * scale
        wk = Wqkv[krows, :]
        wvc = Wqkv[vrows, :]
        wqk = _bf16(np.concatenate([wq, wk], axis=0).T)
        wv = _bf16(wvc.T)
        bqk = np.stack([bqkv[qrows] * scale, bqkv[krows]], axis=1)
        woT = _bf16(Wo[:, 128 * c:128 * (c + 1)].T)
        im = {
            "xT": xTn, "wqk": wqk, "wv": wv,
            "bqk": np.ascontiguousarray(bqk, dtype=np.float32),
            "cosT": cosT, "sinS": sinS, "woT": woT,
        }
        if mode == "bias":
            im["maskT"] = _bf16((mb != 0).astype(np.float32).T)
        in_maps.append(im)
    return in_maps


def kernel(x, mask, Wqkv, bqkv, Wo, bo):
    bqkv = np.asarray(bqkv, dtype=np.float32)
    Wo = np.asarray(Wo, dtype=np.float32)
    bo = np.asarray(bo, dtype=np.float32)

    mode, mb = _detect_mode(mask)
    nc, dram = _build_program(mode)
    in_maps = _prepare_in_maps(x, mask, Wqkv, bqkv, Wo, bo, mode, mb)

    res = run_bass_kernel_spmd(nc, in_maps, core_ids=list(range(NCORES)))
    y = np.zeros((TB, C), dtype=np.float32)
    for c in range(NCORES):
        y += res.results[c]["y"].astype(np.float32)
    bv = bqkv[2 * C:3 * C]
    y += (bo + bv @ Wo.T)[None, :]
    return y.reshape(B, T, C)


# revision 66
# speedup vs baseline: 1.0148x; 1.0148x over previous
"""Trainium2 Bass kernel for CarlosSelfAttention (B=2, T=2048, C=1024, H=16).

Sharding: tensor-parallel over heads. 8 cores x 2 heads each.
Each core computes q/k/v projections for its 2 heads, RoPE, causal
attention, and a partial out-projection against its 128 columns of Wo.
The host sums the 8 partial outputs (the TP all-reduce) and adds the
output bias plus the (v-bias @ Wo.T) correction term.

v2: full bf16 pipeline (matmuls, activations, DMA) with a dense PE
schedule: qkv for batch 1 is interleaved into attention for batch 0 as
"filler" matmuls, and both out-projections are interleaved into
attention for batch 1, so the PE never idles long enough for the HAM
clock gate to re-throttle. V^T is produced directly by x-stationary
matmuls (no PE transposes). All on-chip layouts are "transposed"
([dim, token]) so every matmul contraction lands on the partition axis.
"""

import numpy as np

import concourse.bass as bass
import concourse.tile as tile
from concourse import bacc, mybir
from concourse.bass_utils import run_bass_kernel_spmd

F32 = mybir.dt.float32
BF16 = mybir.dt.bfloat16
AF = mybir.ActivationFunctionType

B, T, C, H, HD = 2, 2048, 1024, 16, 64
NCORES = 8
TB = B * T          # 4096
QCH = 512           # q-chunk (moving dim)
NQC = T // QCH      # 4 q-chunks per batch
NKT = T // 128      # 16 k-tiles per batch
NTC = TB // QCH     # 8 t-chunks for the projections
NCT = C // 128      # 8 contraction tiles
VW = HD + 1         # 65: value dims + ones row for the softmax sum

_PROG_CACHE: dict = {}


def _emit(tc, mode, dram, debug=None):
    nc = tc.nc
    from contextlib import ExitStack
    from collections import deque

    xT, wqk, wv, bqk, cosT, sinS, woT, y = (
        dram["xT"], dram["wqk"], dram["wv"], dram["bqk"], dram["cosT"],
        dram["sinS"], dram["woT"], dram["y"])
    maskT = dram.get("maskT")

    with ExitStack() as ctx:
        constp = ctx.enter_context(tc.tile_pool(name="const", bufs=1))
        pers = ctx.enter_context(tc.tile_pool(name="pers", bufs=1))

        # ---- constants (order matters: first matmul needs bqk+wqk only) ----
        wsb = constp.tile([128, NCT, 256], BF16)
        nc.sync.dma_start(wsb[:, :, 0:128],
                          wqk[:, 0:128].rearrange("(a p) m -> p a m", p=128))
        bqk_sb = constp.tile([128, 2], F32)
        wv_sb = constp.tile([128, NCT, 128], BF16)

        def early_consts():
            nc.sync.dma_start(wsb[:, :, 128:256],
                              wqk[:, 128:256].rearrange("(a p) m -> p a m",
                                                        p=128))
            nc.sync.dma_start(bqk_sb[:], bqk[:])

        # ---- persistent activations ----
        qTb = pers.tile([128, TB], BF16)
        kTb = pers.tile([128, TB], BF16)
        Vsb = [[pers.tile([128, NKT * VW], BF16, name=f"Vsb{b}{h}")
                for h in range(2)] for b in range(B)]
        OTb = [pers.tile([128, T], BF16, name=f"OTb{b}") for b in range(B)]
        for b in range(B):
            for h in range(2):
                nc.gpsimd.memset(Vsb[b][h][:], 1.0)
        id65 = constp.tile([VW, VW], BF16)
        nc.gpsimd.memset(id65[:], 1.0)
        nc.gpsimd.affine_select(
            out=id65[:], in_=id65[:], compare_op=mybir.AluOpType.is_equal,
            fill=0.0, base=0, channel_multiplier=1, pattern=[[-1, VW]])

        # cos/sin + wo land while the first qkv matmuls run (DMAs are
        # emitted inside phase A, after the first x-tile loads)
        cos_sb = constp.tile([128, T], BF16)
        sin_sb = constp.tile([128, T], BF16)
        wo_sb = constp.tile([128, C], BF16)
        wo_lo = constp.tile([64, C], BF16)

        def late_consts():
            nc.sync.dma_start(wv_sb[:],
                              wv[:].rearrange("(a p) m -> p a m", p=128))
            nc.sync.dma_start(cos_sb[:], cosT[:])
            nc.sync.dma_start(sin_sb[:], sinS[:])
            nc.sync.dma_start(wo_sb[:], woT[:])
            nc.sync.dma_start(wo_lo[:], woT[64:128, :])
            if mode == "bias":
                nc.sync.dma_start(
                    mask_sb[:], maskT[:].rearrange("(a p) m -> p a m", p=128))
        if mode == "bias":
            mask_sb = pers.tile([128, NKT, T], BF16)

        def qkv_units(xp, psp, swpp, rtp, qfp, tca, tcb, qtag="q", vtag="v",
                      evict_eng="scalar", extra=None):
            """Units for q/k projection + rope + V^T for t-chunks tca, tcb.
            Returns a list of closures; call in order to emit."""
            units = []
            tsa = slice(tca * QCH, (tca + 1) * QCH)
            tsb = slice(tcb * QCH, (tcb + 1) * QCH)
            xx = {}
            qf = {}

            def load(tci, ts, half):
                def u():
                    if half == 0:
                        xx[tci] = xp.tile([128, NCT, QCH], BF16, tag="x",
                                          name=f"xt{tci}")
                    xt = xx[tci]
                    h = NCT // 2
                    hs = slice(0, h) if half == 0 else slice(h, NCT)
                    nc.sync.dma_start(
                        xt[:, hs, :],
                        xT[hs.start * 128:hs.stop * 128, ts]
                        .rearrange("(a p) m -> p a m", p=128))
                return u

            def mm_g(g, ct, pget):
                def u():
                    psa, psb = pget()
                    w = wsb[:, ct, g * 128:(g + 1) * 128]
                    nc.tensor.matmul(psa[:], w, xx[tca][:, ct, :],
                                     start=(ct == 0), stop=(ct == NCT - 1))
                    nc.tensor.matmul(psb[:], w, xx[tcb][:, ct, :],
                                     start=(ct == 0), stop=(ct == NCT - 1))
                return u

            def evict(g, pget, which, ts, tci):
                def u():
                    ps = pget()[which]
                    qf[(g, tci)] = f = qfp.tile([128, QCH], BF16, tag="qf",
                                                name=f"qf{g}_{tci}")
                    if evict_eng == "scalar":
                        nc.scalar.activation(f[:], ps[:], AF.Identity,
                                             bias=bqk_sb[:, g:g + 1])
                    else:
                        nc.vector.tensor_scalar_add(f[:], ps[:],
                                                    bqk_sb[:, g:g + 1])
                return u

            def rope(g, tci):
                def u():
                    f = qf.pop((g, tci))
                    swp = swpp.tile([128, QCH], BF16, tag="swp",
                                    name=f"swp{g}_{tci}")
                    for o in (0, 64):
                        nc.sync.dma_start(swp[o:o + 32, :],
                                          f[o + 32:o + 64, :])
                        nc.sync.dma_start(swp[o + 32:o + 64, :],
                                          f[o:o + 32, :])
                    dst = qTb if g == 0 else kTb
                    # position within the batch (rope tables are [128, T])
                    ps_ = slice((tci % NQC) * QCH, (tci % NQC + 1) * QCH)
                    gs = slice(tci * QCH, (tci + 1) * QCH)
                    rt = rtp.tile([128, QCH], BF16, tag="rt",
                                  name=f"rt{g}_{tci}")
                    nc.vector.tensor_mul(rt[:], swp[:], sin_sb[:, ps_])
                    nc.vector.tensor_mul(dst[:, gs], f[:], cos_sb[:, ps_])
                    nc.vector.tensor_add(dst[:, gs], dst[:, gs], rt[:])
                return u

            def vtr(tci, sub):
                # V^T for token sub-tile: [128 tokens, 128 dims(2 heads)]
                def u():
                    b, tt = divmod(tci * (QCH // 128) + sub, NKT)
                    psv = psp.tile([128, 128], F32, tag=vtag,
                                   name=f"psv{tci}_{sub}")
                    xt = xx[tci]
                    for ct in range(NCT):
                        nc.tensor.matmul(
                            psv[:], xt[:, ct, sub * 128:(sub + 1) * 128],
                            wv_sb[:, ct, :],
                            start=(ct == 0), stop=(ct == NCT - 1))
                    for h in range(2):
                        nc.vector.tensor_copy(
                            Vsb[b][h][:, tt * VW:tt * VW + HD],
                            psv[:, h * HD:(h + 1) * HD])
                return u

            ps_cache = {}

            def pget_for(g):
                def pget():
                    if g not in ps_cache:
                        ps_cache[g] = (
                            psp.tile([128, QCH], F32, tag=qtag,
                                     name=f"psq{tca}_{g}a"),
                            psp.tile([128, QCH], F32, tag=qtag,
                                     name=f"psq{tca}_{g}b"))
                    return ps_cache[g]
                return pget

            # loads first (deep DMA prefetch), then matmuls
            units.append(load(tca, tsa, 0))
            units.append(load(tcb, tsb, 0))
            if extra is not None:
                units.append(extra)
            units.append(load(tca, tsa, 1))
            units.append(load(tcb, tsb, 1))
            for g in range(2):
                pget = pget_for(g)
                for ct in range(NCT):
                    units.append(mm_g(g, ct, pget))
                units.append(evict(g, pget, 0, tsa, tca))
                units.append(evict(g, pget, 1, tsb, tcb))
            for g in range(2):
                units.append(rope(g, tca))
                units.append(rope(g, tcb))
            for tci in (tca, tcb):
                for sub in range(QCH // 128):
                    units.append(vtr(tci, sub))
            return units

        def proj_units(psp, ybp, b, tts, scalar_stride=0):
            """Out-projection units for token-tiles tts of batch b.
            Every scalar_stride-th eviction goes to ScalarE (0 = never).
            Each tile tt: 2 matmuls -> one merged [128, 1024] store."""
            units = []
            ybs = {}

            def unit(tt, ncol, on_scalar):
                def u():
                    ps = psp.tile([128, QCH], F32, tag="f",
                                  name=f"psy{b}_{tt}_{ncol}")
                    nc.tensor.matmul(
                        ps[:], OTb[b][:, tt * 128:(tt + 1) * 128],
                        wo_sb[:, ncol * QCH:(ncol + 1) * QCH],
                        start=True, stop=True)
                    if ncol == 0:
                        ybs[tt] = ybp.tile([128, C], BF16, tag="yb",
                                           name=f"yb{b}_{tt}")
                    yb = ybs[tt]
                    ys = slice(ncol * QCH, (ncol + 1) * QCH)
                    if on_scalar:
                        nc.scalar.activation(yb[:, ys], ps[:], AF.Copy)
                    else:
                        nc.vector.tensor_copy(yb[:, ys], ps[:])
                    if ncol == 1:
                        nc.sync.dma_start(
                            y[b * T + tt * 128:b * T + (tt + 1) * 128, :],
                            ybs.pop(tt)[:])
                return u
            i = 0
            for tt in tts:
                for ncol in range(2):
                    i += 1
                    units.append(unit(tt, ncol,
                                      scalar_stride and i % scalar_stride == 0))
            return units

        def attn_b(pools, b, fillers, qc_done=None, dbg=None, tail_out=None):
            pss, pso, ptp, smol, bcp = pools
            PIPE = 3
            nks = [4 * (qc + 1) if mode == "causal" else NKT
                   for qc in range(NQC)]
            kts_left = sum(nks)
            for qc in range(NQC):
                nk = nks[qc]
                qs = slice(b * T + qc * QCH, b * T + (qc + 1) * QCH)
                psO0 = pso.tile([VW, QCH], F32, tag="o", name=f"psO0_{b}{qc}")
                psO1 = pso.tile([VW, QCH], F32, tag="o", name=f"psO1_{b}{qc}")
                pts = {}

                def emit_pv(j, nk=nk, psO0=psO0, psO1=psO1, pts=pts):
                    st, sp = (j == 0), (j == nk - 1)
                    pt = pts.pop(j)
                    nc.tensor.matmul(
                        psO0[:], Vsb[b][0][:, j * VW:(j + 1) * VW],
                        pt[:, 0:QCH], start=st, stop=sp)
                    nc.tensor.matmul(
                        psO1[:], Vsb[b][1][:, j * VW:(j + 1) * VW],
                        pt[:, QCH:2 * QCH], start=st, stop=sp)

                for kt in range(nk):
                    ks = slice(b * T + kt * 128, b * T + (kt + 1) * 128)
                    psS = pss.tile([128, 2 * QCH], F32, tag="s",
                                   name=f"psS{b}{qc}{kt}")
                    nc.tensor.matmul(psS[:, 0:QCH], kTb[0:64, ks],
                                     qTb[0:64, qs], start=True, stop=True)
                    nc.tensor.matmul(psS[:, QCH:2 * QCH], kTb[64:128, ks],
                                     qTb[64:128, qs], start=True, stop=True)
                    pt = ptp.tile([128, 2 * QCH], BF16, tag="pt",
                                  name=f"pt{b}{qc}{kt}")
                    nc.scalar.activation(pt[:], psS[:], AF.Exp)
                    if mode == "causal" and kt >= 4 * qc:
                        base = qc * QCH - kt * 128
                        ptv = pt[:].rearrange("p (h q) -> p h q", q=QCH)
                        nc.gpsimd.affine_select(
                            out=ptv, in_=ptv,
                            compare_op=mybir.AluOpType.is_ge,
                            fill=0.0, base=base, channel_multiplier=-1,
                            pattern=[[0, 2], [1, QCH]])
                    elif mode == "bias":
                        mt = mask_sb[:, kt, qc * QCH:(qc + 1) * QCH]
                        nc.vector.tensor_mul(pt[:, 0:QCH], pt[:, 0:QCH], mt)
                        nc.vector.tensor_mul(pt[:, QCH:2 * QCH],
                                             pt[:, QCH:2 * QCH], mt)
                    pts[kt] = pt
                    if dbg is not None and qc == 0 and kt == 0:
                        nc.vector.tensor_copy(dbg["pt"][:], pt[:])
                    # pace the fillers across ALL remaining k-tiles
                    want = (len(fillers) + kts_left - 1) // kts_left \
                        if fillers else 0
                    kts_left -= 1
                    for _ in range(min(want, 4)):
                        if fillers:
                            fillers.popleft()()
                    if kt >= PIPE:
                        emit_pv(kt - PIPE)
                for j in range(max(0, nk - PIPE), nk):
                    emit_pv(j)

                # fast-evict psO to SBUF (frees the PSUM banks in one DVE
                # round-trip), then normalize from the SBUF copy
                oqs = slice(qc * QCH, (qc + 1) * QCH)
                nm = f"{b}{qc}"
                if dbg is not None and qc == 0:
                    nc.vector.tensor_copy(dbg["po"][0:VW, :], psO0[:])
                if tail_out is not None and qc == NQC - 1:
                    # Final q-chunk: skip normalize/OTb entirely.  Evict the
                    # unnormalized output (bf16) + per-token reciprocal sums;
                    # the tail projection folds the softmax normalization in
                    # as a per-token (per-partition) scale.
                    occ = smol.tile([VW, 2 * QCH], BF16, tag="occ",
                                    name="octail")
                    nc.scalar.activation(occ[0:VW, 0:QCH], psO0[:], AF.Copy)
                    nc.scalar.activation(occ[0:VW, QCH:2 * QCH], psO1[:],
                                         AF.Copy)
                    tail_out["oc"] = occ
                    if qc_done is not None:
                        qc_done(qc)
                    continue
                oc = smol.tile([VW, 2 * QCH], F32, tag="oc", name=f"oc{nm}")
                if qc == NQC - 1:
                    # scalar is idle at the window end; keep DVE clear for
                    # the latency-critical chain to the final projections
                    nc.scalar.activation(oc[0:VW, 0:QCH], psO0[:], AF.Copy)
                    nc.scalar.activation(oc[0:VW, QCH:2 * QCH], psO1[:],
                                         AF.Copy)
                else:
                    nc.vector.tensor_copy(oc[0:VW, 0:QCH], psO0[:])
                    nc.vector.tensor_copy(oc[0:VW, QCH:2 * QCH], psO1[:])
                rz = smol.tile([1, 2 * QCH], F32, tag="rz", name=f"rz{nm}")
                nc.gpsimd.dma_start(rz[:], oc[64:65, :])
                rr = smol.tile([1, 2 * QCH], F32, tag="rr", name=f"rr{nm}")
                nc.vector.reciprocal_approx_fast(rr[:], rz[:])
                bc = bcp.tile([64, 2 * QCH], F32, tag="bc", name=f"bc{nm}")
                nc.gpsimd.partition_broadcast(bc[:], rr[:])
                if dbg is not None and qc == 0:
                    nc.vector.tensor_copy(dbg["nr"][0:64, :], bc[:])
                    nc.vector.tensor_copy(dbg["nr2"][0:1, :], rr[:])
                    nc.vector.tensor_copy(dbg["nr2"][64:65, :], oc[64:65, :])
                nc.vector.tensor_mul(OTb[b][0:64, oqs], oc[0:64, 0:QCH],
                                     bc[:, 0:QCH])
                otmp = bcp.tile([64, QCH], BF16, tag="otmp", name=f"ot{nm}")
                nc.vector.tensor_mul(otmp[:], oc[0:64, QCH:2 * QCH],
                                     bc[:, QCH:2 * QCH])
                nc.gpsimd.dma_start(OTb[b][64:128, oqs], otmp[:])
                if qc_done is not None:
                    qc_done(qc)
            while fillers:
                fillers.popleft()()

        # ---- phase A: qkv + rope + V^T for batch 0 ----
        with tc.tile_pool(name="xp", bufs=5) as xp, \
             tc.tile_pool(name="qfp", bufs=6) as qfp, \
             tc.tile_pool(name="swp", bufs=4) as swpp, \
             tc.tile_pool(name="rtp", bufs=4) as rtp:
            with tc.tile_pool(name="psA", bufs=4, space="PSUM") as psA:
                def phaseA_consts():
                    early_consts()
                    late_consts()
                for u in qkv_units(xp, psA, swpp, rtp, qfp, 0, 1,
                                   extra=phaseA_consts):
                    u()
                for u in qkv_units(xp, psA, swpp, rtp, qfp, 2, 3):
                    u()

            # ---- phases B/C: attention with fillers ----
            with tc.tile_pool(name="pss", bufs=2, space="PSUM") as pss, \
                 tc.tile_pool(name="pso", bufs=2, space="PSUM") as pso, \
                 tc.tile_pool(name="psf", bufs=2, space="PSUM") as psf, \
                 tc.tile_pool(name="ptp", bufs=8) as ptp, \
                 tc.tile_pool(name="smol", bufs=2) as smol, \
                 tc.tile_pool(name="bcp", bufs=2) as bcp, \
                 tc.tile_pool(name="ybp", bufs=4) as ybp:
                dbg = None
                if debug is not None:
                    dbg = {"pt": pers.tile([128, 2 * QCH], BF16, name="dbgpt"),
                           "po": pers.tile([128, QCH], F32, name="dbgpo"),
                           "nr": pers.tile([128, 2 * QCH], F32, name="dbgnr"),
                           "nr2": pers.tile([128, 2 * QCH], F32,
                                            name="dbgnr2")}
                fillers = deque()
                fillers.extend(qkv_units(xp, psf, swpp, rtp, qfp, 4, 5,
                                         qtag="f", vtag="f",
                                         evict_eng="vector"))
                fillers.extend(qkv_units(xp, psf, swpp, rtp, qfp, 6, 7,
                                         qtag="f", vtag="f",
                                         evict_eng="vector"))
                attn_b((pss, pso, ptp, smol, bcp), 0, fillers, dbg=dbg)

                fillers = deque()
                fillers.extend(proj_units(psf, ybp, 0, range(NKT - 2),
                                          scalar_stride=4))

                def qc_done(qc):
                    if qc > 0:
                        fillers.extend(proj_units(
                            psf, ybp, 1, range(4 * (qc - 1), 4 * qc),
                            scalar_stride=4))
                tail = {}
                attn_b((pss, pso, ptp, smol, bcp), 1, fillers, qc_done,
                       tail_out=tail)
                # held-back b0 tiles fill the PE while the tail sums/recip
                # chain completes
                for u in proj_units(psf, ybp, 0, range(NKT - 2, NKT),
                                    scalar_stride=2):
                    u()
                # extract per-token softmax sums via tiny PE transposes,
                # then reciprocal on token partitions
                occ = tail["oc"]
                rsf = smol.tile([128, 8], F32, tag="rsf", name="rsftail")
                for o in range(8):
                    pst = psf.tile([128, VW], BF16, tag="f", name=f"pst{o}")
                    nc.tensor.transpose(pst[:], occ[0:VW, o * 128:(o + 1) * 128],
                                        id65[:])
                    nc.vector.tensor_copy(rsf[:, o:o + 1], pst[:, 64:65])
                rc = smol.tile([128, 8], F32, tag="rc", name="rctail")
                nc.vector.reciprocal_approx_fast(rc[:], rsf[:])
                qc0 = NQC - 1
                for tr in range(4):
                    tt = 4 * qc0 + tr
                    ts0 = slice(tr * 128, (tr + 1) * 128)
                    ts1 = slice(QCH + tr * 128, QCH + (tr + 1) * 128)
                    ybt = ybp.tile([128, C], BF16, tag="yb", name=f"ybt{tt}")
                    # borrow the dead attention PSUM slots (pss/pso) so two
                    # token-tiles pipeline through the tail
                    ps0 = pss.tile([128, C], F32, tag="s", name=f"tps0_{tt}")
                    th = bcp.tile([128, C], F32, tag="th", name=f"th{tt}")
                    for ncol in range(2):
                        ys = slice(ncol * QCH, (ncol + 1) * QCH)
                        nc.tensor.matmul(ps0[:, ys], occ[0:64, ts0],
                                         wo_sb[0:64, ys],
                                         start=True, stop=True)
                        ps1 = psf.tile([128, QCH], F32, tag="f",
                                       name=f"tps1_{tt}_{ncol}")
                        nc.tensor.matmul(ps1[:], occ[0:64, ts1],
                                         wo_lo[0:64, ys],
                                         start=True, stop=True)
                        nc.scalar.activation(th[:, ys], ps1[:], AF.Copy,
                                             scale=rc[:, 4 + tr:5 + tr])
                    nc.vector.scalar_tensor_tensor(
                        ybt[:], ps0[:], rc[:, tr:tr + 1], th[:],
                        mybir.AluOpType.mult, mybir.AluOpType.add)
                    nc.sync.dma_start(
                        y[T + tt * 128:T + (tt + 1) * 128, :], ybt[:])

        if debug is not None:
            nc.sync.dma_start(debug["dbg_pt"][:], dbg["pt"][:])
            nc.sync.dma_start(debug["dbg_po"][:], dbg["po"][:])
            nc.sync.dma_start(debug["dbg_nr"][:], dbg["nr"][:])
            nc.sync.dma_start(debug["dbg_nr2"][:], dbg["nr2"][:])
            nc.sync.dma_start(debug["dbg_q"][:], qTb[:])
            nc.sync.dma_start(debug["dbg_k"][:], kTb[:])
            nc.sync.dma_start(debug["dbg_v00"][:], Vsb[0][0][:])
            nc.sync.dma_start(debug["dbg_v01"][:], Vsb[1][0][:])
            nc.sync.dma_start(debug["dbg_o0"][:], OTb[0][:])
            nc.sync.dma_start(debug["dbg_o1"][:], OTb[1][:])


def _build_program(mode):
    if mode in _PROG_CACHE:
        return _PROG_CACHE[mode]
    nc = bacc.Bacc("TRN2", target_bir_lowering=False, debug=False,
                   num_devices=NCORES)
    dram = {
        "xT": nc.dram_tensor("xT", [C, TB], BF16, kind="ExternalInput").ap(),
        "wqk": nc.dram_tensor("wqk", [C, 256], BF16, kind="ExternalInput").ap(),
        "wv": nc.dram_tensor("wv", [C, 128], BF16, kind="ExternalInput").ap(),
        "bqk": nc.dram_tensor("bqk", [128, 2], F32, kind="ExternalInput").ap(),
        "cosT": nc.dram_tensor("cosT", [128, T], BF16,
                               kind="ExternalInput").ap(),
        "sinS": nc.dram_tensor("sinS", [128, T], BF16,
                               kind="ExternalInput").ap(),
        "woT": nc.dram_tensor("woT", [128, C], BF16, kind="ExternalInput").ap(),
        "y": nc.dram_tensor("y", [TB, C], BF16, kind="ExternalOutput").ap(),
    }
    if mode == "bias":
        dram["maskT"] = nc.dram_tensor("maskT", [T, T], BF16,
                                       kind="ExternalInput").ap()
    with tile.TileContext(nc) as tc:
        _emit(tc, mode, dram)
    nc.compile()
    _PROG_CACHE[mode] = (nc, dram)
    return nc, dram


def _rope_tables():
    inv_freq = 1.0 / (10000.0 ** (np.arange(0, HD, 2, dtype=np.float64) / HD))
    freqs = np.arange(T, dtype=np.float64)[:, None] * inv_freq[None, :]
    cos = np.concatenate([np.cos(freqs), np.cos(freqs)], axis=-1)  # [T, 64]
    sin = np.concatenate([np.sin(freqs), np.sin(freqs)], axis=-1)
    cE = cos[:, 0::2].T  # [32, T] rows i -> dim 2i
    cO = cos[:, 1::2].T
    sE = sin[:, 0::2].T
    sO = sin[:, 1::2].T
    cosT = np.concatenate([cE, cO, cE, cO], axis=0)
    sinS = np.concatenate([-sE, sO, -sE, sO], axis=0)
    return cosT, sinS


def _bf16(a):
    import ml_dtypes
    return np.ascontiguousarray(a.astype(ml_dtypes.bfloat16))


def _detect_mode(mask):
    mb = np.asarray(mask).reshape(T, T)
    if np.array_equal(mb != 0, np.tril(np.ones((T, T), dtype=bool))):
        return "causal", mb
    if np.all(mb != 0):
        return "dense", mb
    return "bias", mb


def _prepare_in_maps(x, mask, Wqkv, bqkv, Wo, bo, mode, mb):
    x = np.asarray(x, dtype=np.float32)
    Wqkv = np.asarray(Wqkv, dtype=np.float32)
    bqkv = np.asarray(bqkv, dtype=np.float32)
    Wo = np.asarray(Wo, dtype=np.float32)

    xTn = _bf16(x.reshape(TB, C).T)
    cosT, sinS = _rope_tables()
    cosT, sinS = _bf16(cosT), _bf16(sinS)
    scale = 1.0 / np.sqrt(np.float32(HD))

    evens = np.arange(0, HD, 2)
    odds = evens + 1

    in_maps = []
    for c in range(NCORES):
        h0, h1 = 2 * c, 2 * c + 1
        qrows = np.concatenate([h0 * HD + evens, h0 * HD + odds,
                                h1 * HD + evens, h1 * HD + odds])
        krows = C + qrows
        vrows = np.concatenate([2 * C + h0 * HD + np.arange(HD),
                                2 * C + h1 * HD + np.arange(HD)])
        wq = Wqkv[qrows, :] * scale
        wk = Wqkv[krows, :]
        wvc = Wqkv[vrows, :]
        wqk = _bf16(np.concatenate([wq, wk], axis=0).T)
        wv = _bf16(wvc.T)
        bqk = np.stack([bqkv[qrows] * scale, bqkv[krows]], axis=1)
        woT = _bf16(Wo[:, 128 * c:128 * (c + 1)].T)
        im = {
            "xT": xTn, "wqk": wqk, "wv": wv,
            "bqk": np.ascontiguousarray(bqk, dtype=np.float32),
            "cosT": cosT, "sinS": sinS, "woT": woT,
        }
        if mode == "bias":
            im["maskT"] = _bf16((mb != 0).astype(np.float32).T)
        in_maps.append(im)
    return in_maps


def kernel(x, mask, Wqkv, bqkv, Wo, bo):
    bqkv = np.asarray(bqkv, dtype=np.float32)
    Wo = np.asarray(Wo, dtype=np.float32)
    bo = np.asarray(bo, dtype=np.float32)

    mode, mb = _detect_mode(mask)
    nc, dram = _build_program(mode)
    in_maps = _prepare_in_maps(x, mask, Wqkv, bqkv, Wo, bo, mode, mb)

    res = run_bass_kernel_spmd(nc, in_maps, core_ids=list(range(NCORES)))
    y = np.zeros((TB, C), dtype=np.float32)
    for c in range(NCORES):
        y += res.results[c]["y"].astype(np.float32)
    bv = bqkv[2 * C:3 * C]
    y += (bo + bv @ Wo.T)[None, :]
    return y.reshape(B, T, C)
